# revision 31
# baseline (speedup 1.0000x reference)
"""GQA prefill attention (B=1, T=2048, DIM=4096, 32 q-heads / 8 kv-heads),
tensor-parallel over 8 NeuronCores.

Sharding: core c owns kv head c and its 4 query heads: wq rows
[512c, 512c+512), wk/wv rows [128c, 128c+128), wo cols [512c, 512c+512).
Each core computes a partial y = attn_c @ wo_c.T in [T, DIM]; the host sums
the 8 partials (the "all-reduce after wo").

v5 design (all matmul operands bf16, PSUM f32, single pipeline over
t-chunks j: proj -> attention(4 heads) -> out-proj):
  - Host pre-lays x and wq/wk/wv out as SBUF images so every DMA line is
    contiguous per partition (2-4KB lines instead of 1KB strided): the
    v4 trace showed ~30us of weight-DMA starvation in chunk 0.
  - proj d-loop runs on d-PAIRS: one [128,2,512] x DMA feeds 12 matmuls
    (fewer DMAs, fewer semaphore waits on the PE queue - unsatisfied
    waits break LDWEIGHTS prefetch and turn 216ns slots into ~310ns).
  - Causal masking by matmul: diagonal tiles get one N=128 matmul that
    accumulates -C*relu(s-t) into the S psum; exp underflows to 0.
  - exp pair-batched: S pairs live in [128,2,512] two-bank psum tiles,
    ONE activation per pair: (1024+352)/1.2 ns vs 2*(512+352)/1.2.
  - Softmax denominator: clean (off-diagonal) s-tiles are summed in
    bf16 on the DVE as quad trees (2 adds per 4 tiles) with a single
    ones-matmul per quad; diagonal tiles use per-tile partial-width
    ones-matmuls. All accumulate into one l psum per (head, chunk).
  - The pair loop is software-pipelined (S of pair g+1 before rowsum/O
    of pair g); head finish chains (recip -> PE broadcast -> normalize)
    are emitted early in the next head.
"""

import sys

sys.path.insert(0, "/opt/trn_rl_repo")

import ml_dtypes
import numpy as np

import concourse.bass as bass
import concourse.tile as tile
from concourse import bacc, mybir
from concourse.bass_utils import run_bass_kernel_spmd
from concourse.masks import make_identity

T = 2048
DIM = 4096
HD = 128
NCORE = 8
NH = 4  # q heads per core
TCH = 512
NTCH = T // TCH  # 4 t-chunks
NST = T // 128  # 16 s-tiles
NDT = DIM // 128  # 32 d-tiles
NDP = NDT // 2  # 16 d-pairs
F32 = mybir.dt.float32
BF16 = mybir.dt.bfloat16
SCALE = 1.0 / float(np.sqrt(HD))
MASKC = 8192.0  # big, bf16-exact; C*1 dwarfs any |score| here

# test.py can flip these before calling kernel() to get profiling info
TRACE = False
LAST = {}

_CACHE = {}


def _build():
    nc = bacc.Bacc("TRN2", target_bir_lowering=False, debug=False, num_devices=NCORE)
    # host-prepared SBUF images: per-partition-contiguous DMA lines
    x2 = nc.dram_tensor("x2", [128, NTCH, NDT, TCH], BF16, kind="ExternalInput").ap()
    wq2 = nc.dram_tensor("wq2", [128, NDT, NH * HD], BF16, kind="ExternalInput").ap()
    wk2 = nc.dram_tensor("wk2", [128, NDT, HD], BF16, kind="ExternalInput").ap()
    wv2 = nc.dram_tensor("wv2", [128, NDT, HD], BF16, kind="ExternalInput").ap()
    woT = nc.dram_tensor("woT", [NH * HD, DIM], BF16, kind="ExternalInput").ap()
    ones_in = nc.dram_tensor("ones", [128, 1], BF16, kind="ExternalInput").ap()
    onescol_in = nc.dram_tensor("onescol", [1, 128], BF16, kind="ExternalInput").ap()
    maskA_in = nc.dram_tensor("maskA", [128, 128], BF16, kind="ExternalInput").ap()
    maskB_in = nc.dram_tensor("maskB", [128, 128], BF16, kind="ExternalInput").ap()
    y = nc.dram_tensor("y", [T, DIM], BF16, kind="ExternalOutput").ap()

    wor = woT.rearrange("(hb p) f -> p hb f", p=128)

    with tile.TileContext(nc) as tc:
        with (
            tc.tile_pool(name="persist", bufs=1) as persist,
            tc.tile_pool(name="xs", bufs=9) as xs,
            tc.tile_pool(name="phs", bufs=7) as phs,
            tc.tile_pool(name="pss", bufs=3) as pss,
            tc.tile_pool(name="vts", bufs=2) as vts,
            tc.tile_pool(name="recips", bufs=2) as recips,
            tc.tile_pool(name="recipbs", bufs=2) as recipbs,
            tc.tile_pool(name="rbcs", bufs=2) as rbcs,
            tc.tile_pool(name="ys", bufs=6) as ys,
        ):
            qt_sb = [persist.tile([128, T], BF16, tag=f"qt{h}", name=f"qt{h}") for h in range(NH)]
            kt_sb = persist.tile([128, T], BF16, tag="kt")
            v_sb = persist.tile([128, NST, HD], BF16, tag="v")
            ao_sb = [persist.tile([128, TCH], BF16, tag=f"ao{h}", name=f"ao{h}") for h in range(NH)]
            wq_sb = persist.tile([128, NDT, NH * HD], BF16, tag="wq")
            wk_sb = persist.tile([128, NDT, HD], BF16, tag="wk")
            wv_sb = persist.tile([128, NDT, HD], BF16, tag="wv")
            wo_sb = persist.tile([128, NH, DIM], BF16, tag="wo")
            ones_sb = persist.tile([128, 1], BF16, tag="ones")
            onescol = persist.tile([1, 128], BF16, tag="onescol")
            maskA = persist.tile([128, 128], BF16, tag="maskA")
            maskB = persist.tile([128, 128], BF16, tag="maskB")
            ident = persist.tile([128, 128], BF16, tag="ident")

            pend = {}  # h -> (psum_l, psum_ot)

            def finish_head(h):
                """recip -> gpsimd partition broadcast -> normalize into ao_sb[h]."""
                psum_l, psum_ot = pend.pop(h)
                recip = recips.tile([1, TCH], F32, tag="recip")
                nc.vector.reciprocal_approx_fast(recip[:], psum_l[:])
                rbc = rbcs.tile([128, TCH], F32, tag="rbc")
                nc.gpsimd.partition_broadcast(rbc[:], recip[:])
                nc.vector.tensor_mul(ao_sb[h][:], psum_ot[:], rbc[:])

            for j in range(NTCH):
                cs = slice(j * TCH, (j + 1) * TCH)
                # ---------------- projections for chunk j ----------------
                with tc.tile_pool(name=f"pj{j}", bufs=1, space="PSUM") as psp:
                    qps = [
                        psp.tile([128, TCH], F32, tag=f"projq{fq}", name=f"projq{fq}")
                        for fq in range(NH)
                    ]
                    kps = psp.tile([128, TCH], F32, tag="projk")
                    vps = psp.tile([128, TCH], F32, tag="projv")
                    for dp in range(NDP):
                        if j == 0:
                            # weight streams: wq on the scalar queue with a
                            # 2-pair lookahead, wk/wv/wo on the gpsimd
                            # queue (all per-partition contiguous now)
                            if dp == 0:
                                nc.gpsimd.dma_start(out=wk_sb[:, 0:4, :], in_=wk2[:, 0:4, :])
                                nc.gpsimd.dma_start(out=wv_sb[:, 0:4, :], in_=wv2[:, 0:4, :])
                                for lp_ in (0, 1, 2):
                                    g = slice(2 * lp_, 2 * lp_ + 2)
                                    nc.scalar.dma_start(out=wq_sb[:, g, :], in_=wq2[:, g, :])
                                for lp_ in (3, 4):
                                    g = slice(2 * lp_, 2 * lp_ + 2)
                                    nc.gpsimd.dma_start(out=wq_sb[:, g, :], in_=wq2[:, g, :])
                            elif dp == 1:
                                nc.gpsimd.dma_start(out=wk_sb[:, 4:, :], in_=wk2[:, 4:, :])
                                nc.gpsimd.dma_start(out=wv_sb[:, 4:, :], in_=wv2[:, 4:, :])
                                nc.sync.dma_start(out=ones_sb, in_=ones_in)
                                nc.sync.dma_start(out=onescol, in_=onescol_in)
                                nc.sync.dma_start(out=maskA, in_=maskA_in)
                                nc.sync.dma_start(out=maskB, in_=maskB_in)
                            if 1 <= dp <= 12 and dp + 4 < NDP:
                                g = slice(2 * (dp + 4), 2 * (dp + 4) + 2)
                                nc.scalar.dma_start(out=wq_sb[:, g, :], in_=wq2[:, g, :])
                            if dp == 3:
                                make_identity(nc, ident)
                                # warm the exp table well before the first exp
                                expwarm = persist.tile([1, 2], F32, tag="expwarm")
                                nc.vector.memset(expwarm, 0.0)
                                nc.scalar.activation(
                                    out=expwarm[:],
                                    in_=expwarm[:],
                                    func=mybir.ActivationFunctionType.Exp,
                                    scale=1.0,
                                )
                            elif dp == 15:
                                for hb in range(NH):
                                    nc.gpsimd.dma_start(
                                        out=wo_sb[:, hb, :], in_=wor[:, hb, :]
                                    )
                        xbf = xs.tile([128, 2, TCH], BF16, tag="xbf")
                        nc.sync.dma_start(out=xbf, in_=x2[:, j, 2 * dp : 2 * dp + 2, :])
                        for half in range(2):
                            d = 2 * dp + half
                            st = d == 0
                            sp = d == NDT - 1
                            if sp:
                                # last slice: q0 stops first (its copy gates
                                # attention), then k, v, then q1..3
                                order = ["q0", "k", "v", "q1", "q2", "q3"]
                            else:
                                order = ["q0", "q1", "q2", "q3", "k", "v"]
                            for op in order:
                                if op == "k":
                                    nc.tensor.matmul(
                                        kps[:], wk_sb[:, d, :], xbf[:, half, :],
                                        start=st, stop=sp,
                                    )
                                elif op == "v":
                                    nc.tensor.matmul(
                                        vps[:], wv_sb[:, d, :], xbf[:, half, :],
                                        start=st, stop=sp,
                                    )
                                else:
                                    fq = int(op[1])
                                    nc.tensor.matmul(
                                        qps[fq][:],
                                        wq_sb[:, d, fq * HD : (fq + 1) * HD],
                                        xbf[:, half, :],
                                        start=st,
                                        stop=sp,
                                    )
                    # qt0 gates attention's first S-matmul for j>=1; the
                    # V transposes gate the PE queue, so vt comes first on DVE
                    nc.scalar.copy(qt_sb[0][:, cs], qps[0][:])
                    vt_tmp = vts.tile([128, TCH], BF16, tag="vt")
                    nc.vector.tensor_copy(vt_tmp[:], vps[:])
                    nc.vector.tensor_copy(kt_sb[:, cs], kps[:])
                    nc.scalar.copy(qt_sb[2][:, cs], qps[2][:])
                    nc.vector.tensor_copy(qt_sb[1][:, cs], qps[1][:])
                    nc.vector.tensor_copy(qt_sb[3][:, cs], qps[3][:])

                # ---------------- attention + out-proj for chunk j ----------------
                n_i = 4 * j + 4
                with (
                    tc.tile_pool(name=f"ot{j}", bufs=2, space="PSUM") as otp,
                    tc.tile_pool(name=f"lp{j}", bufs=1, space="PSUM") as lp,
                ):
                    retq = []  # cross-head retire queue (2 deep)
                    with tc.tile_pool(name=f"st{j}", bufs=2, space="PSUM") as stp:
                        for h in range(NH):
                            psum_l = lp.tile([1, TCH], F32, tag="l", name=f"l{h}")
                            psum_ot = otp.tile([128, TCH], F32, tag="ot", name=f"ot{h}")
                            pend[h] = (psum_l, psum_ot)
                            state = {"rs_started": False, "quadq": []}

                            def emit_retire(ph2, ia, state=state, psum_l=psum_l, psum_ot=psum_ot):
                                """O-matmuls for pair (ia, ia+1) + denominator work."""
                                diag = (ia + 1) - 4 * j >= 0
                                for half, i in enumerate((ia, ia + 1)):
                                    r = i - 4 * j
                                    c0 = 128 * r if r > 0 else 0
                                    nc.tensor.matmul(
                                        psum_ot[:, c0:],
                                        v_sb[:, i, :],
                                        ph2[:, half, c0:],
                                        start=(i == 0),
                                        stop=(i == n_i - 1),
                                    )
                                    if diag:
                                        nc.tensor.matmul(
                                            psum_l[:, c0:],
                                            ones_sb[:],
                                            ph2[:, half, c0:],
                                            start=not state["rs_started"],
                                            stop=(i == n_i - 1),
                                        )
                                        state["rs_started"] = True
                                if not diag:
                                    state["quadq"].append(ph2)
                                    if len(state["quadq"]) == 2:
                                        pa, pb = state["quadq"]
                                        state["quadq"] = []
                                        ps2 = pss.tile([128, 2, TCH], BF16, tag="ps2")
                                        nc.vector.tensor_add(ps2[:], pa[:], pb[:])
                                        ps1 = pss.tile([128, TCH], BF16, tag="ps1")
                                        nc.vector.tensor_add(
                                            ps1[:], ps2[:, 0, :], ps2[:, 1, :]
                                        )
                                        nc.tensor.matmul(
                                            psum_l[:],
                                            ones_sb[:],
                                            ps1[:],
                                            start=not state["rs_started"],
                                            stop=False,
                                        )
                                        state["rs_started"] = True

                            for g in range(n_i // 2):
                                ia = 2 * g
                                st2 = stp.tile([128, 2, TCH], F32, tag="st")
                                for half, i in enumerate((ia, ia + 1)):
                                    r = i - 4 * j
                                    c0 = 128 * r if r > 0 else 0
                                    nc.tensor.matmul(
                                        st2[:, half, c0:],
                                        kt_sb[:, i * 128 : (i + 1) * 128],
                                        qt_sb[h][:, j * TCH + c0 : (j + 1) * TCH],
                                        start=True,
                                        stop=(r < 0),
                                    )
                                    if r >= 0:  # diagonal: psum += -C*relu(s-t)
                                        nc.tensor.matmul(
                                            st2[:, half, c0 : c0 + 128],
                                            maskA[:],
                                            maskB[:],
                                            start=False,
                                            stop=True,
                                            skip_group_check=True,
                                        )
                                ph2 = phs.tile([128, 2, TCH], BF16, tag="phat")
                                nc.scalar.activation(
                                    out=ph2[:],
                                    in_=st2[:],
                                    func=mybir.ActivationFunctionType.Exp,
                                    scale=SCALE,
                                )
                                if h == 0 and g == 0:
                                    # V transposes for this chunk, in the
                                    # first exp's shadow; needed only by the
                                    # diagonal retires much later
                                    for ii in range(4):
                                        ptrt = lp.tile([128, HD], BF16, tag="tr")
                                        nc.tensor.transpose(
                                            ptrt[:],
                                            vt_tmp[:, ii * 128 : (ii + 1) * 128],
                                            ident[:],
                                        )
                                        nc.vector.tensor_copy(
                                            v_sb[:, 4 * j + ii, :], ptrt[:]
                                        )
                                retq.append((emit_retire, ph2, ia))
                                if len(retq) > 2:
                                    fn, p_, a_ = retq.pop(0)
                                    fn(p_, a_)
                                if g == 1 and h > 0:
                                    finish_head(h - 1)
                        for fn, p_, a_ in retq:
                            fn(p_, a_)

                    # out-proj for chunk j, interleaved with head 3's finish
                    with tc.tile_pool(name=f"psy{j}", bufs=4, space="PSUM") as psy:
                        py0 = psy.tile([128, 512], F32, tag="py", name="py0")
                        py1 = psy.tile([128, 512], F32, tag="py", name="py1")
                        for fc, py in ((0, py0), (1, py1)):
                            for hb in range(3):
                                nc.tensor.matmul(
                                    py[:],
                                    ao_sb[hb][:, 0:128],
                                    wo_sb[:, hb, fc * 512 : (fc + 1) * 512],
                                    start=(hb == 0),
                                    stop=False,
                                )
                        finish_head(3)
                        pre = {0: py0, 1: py1}
                        for tt4 in range(4):
                            tloc = slice(tt4 * 128, (tt4 + 1) * 128)
                            tsl = slice(j * TCH + tt4 * 128, j * TCH + (tt4 + 1) * 128)
                            for fc in range(8):
                                fsl = slice(fc * 512, (fc + 1) * 512)
                                if tt4 == 0 and fc in pre:
                                    py = pre.pop(fc)
                                    hbs = [3]  # 0..2 already accumulated above
                                else:
                                    py = psy.tile([128, 512], F32, tag="py")
                                    hbs = [0, 1, 2, 3]
                                for hb in hbs:
                                    nc.tensor.matmul(
                                        py[:],
                                        ao_sb[hb][:, tloc],
                                        wo_sb[:, hb, fsl],
                                        start=(hb == 0),
                                        stop=(hb == 3),
                                    )
                                yt = ys.tile([128, 512], BF16, tag="yt")
                                if fc % 2 == 0:
                                    nc.vector.tensor_copy(yt[:], py[:])
                                    nc.sync.dma_start(out=y[tsl, fsl], in_=yt[:])
                                else:
                                    nc.scalar.copy(yt[:], py[:])
                                    nc.scalar.dma_start(out=y[tsl, fsl], in_=yt[:])

    nc.compile()
    return nc


def kernel(x, wq, wk, wv, wo):
    x = np.asarray(x, dtype=np.float32)
    wq = np.asarray(wq, dtype=np.float32)
    wk = np.asarray(wk, dtype=np.float32)
    wv = np.asarray(wv, dtype=np.float32)
    wo = np.asarray(wo, dtype=np.float32)

    if "nc" not in _CACHE:
        _CACHE["nc"] = _build()
    nc = _CACHE["nc"]

    # x SBUF image: x2[p, ch, d, c] = x[0][ch*512+c, d*128+p]
    x2 = (
        x[0]
        .reshape(NTCH, TCH, NDT, 128)
        .transpose(3, 0, 2, 1)
        .astype(ml_dtypes.bfloat16)
    )
    x2 = np.ascontiguousarray(x2)

    def wimage(w_rows):  # [F, DIM] -> [128, NDT, F] with [p, d, f] = w[f, d*128+p]
        F = w_rows.shape[0]
        return np.ascontiguousarray(
            w_rows.T.reshape(NDT, 128, F).transpose(1, 0, 2)
        ).astype(ml_dtypes.bfloat16)

    ones = np.ones((128, 1), ml_dtypes.bfloat16)
    onescol = np.ones((1, 128), ml_dtypes.bfloat16)
    idx = np.arange(128)
    maskA = (idx[None, :] >= idx[:, None]).astype(ml_dtypes.bfloat16)  # [m,s]: s>=m
    maskB = ((idx[:, None] > idx[None, :]) * -MASKC).astype(ml_dtypes.bfloat16)  # [m,t]
    in_maps = []
    for c in range(NCORE):
        qs = slice(c * NH * HD, (c + 1) * NH * HD)
        ks = slice(c * HD, (c + 1) * HD)
        in_maps.append(
            {
                "x2": x2,
                "wq2": wimage(wq[qs, :]),
                "wk2": wimage(wk[ks, :]),
                "wv2": wimage(wv[ks, :]),
                "woT": np.ascontiguousarray(wo[:, qs].T).astype(ml_dtypes.bfloat16),
                "ones": ones,
                "onescol": onescol,
                "maskA": maskA,
                "maskB": maskB,
            }
        )

    res = run_bass_kernel_spmd(
        nc, in_maps, core_ids=list(range(NCORE)), trace=TRACE
    )
    LAST["results"] = res

    out = np.zeros((T, DIM), dtype=np.float64)
    for c in range(NCORE):
        out += res.results[c]["y"].astype(np.float64)
    return out.astype(np.float32).reshape(1, T, DIM)
